# revision 27
# baseline (speedup 1.0000x reference)
"""Block-sparse attention Trainium2 kernel (v3, bf16 + A/B pipelined bodies).

Reference: nn.MultiheadAttention-style block-sparse attention, B=1, L=4096,
D=1024, H=16, head_dim=64, block=128, global blocks {0, 24}.

Sharding: head-parallel across 8 cores (2 heads/core); host sums the 8
partial out-projections. All matmul operands are bf16 (PSUM accumulates
f32), which halves HBM traffic (x in, partial-out store) and runs narrow
(128-wide) matmuls at 1 cycle/row. Attention-value products are computed
in transposed form (outT = v_aug.T @ expT); softmax denominators ride
along as row 64 of the augmented V. For steady-state timing the loop body
is unrolled 2x over two independent buffer sets (A/B) so consecutive
bodies pipeline with no WAR hazards on q/k/v. The Act engine runs only
Exp (no act-table swaps); PSUM->SBUF copies are spread over DVE/Pool.
"""

import sys

sys.path.insert(0, "/opt/trn_rl_repo")
import numpy as np

D = 1024
L = 4096
H = 16
HD = 64
NB = 32
GLOB = (0, 24)
P = 128
SCALE = 1.0 / 8.0

_CACHE = {}


def _build_nc(reps=1):
    import contextlib

    import concourse.mybir as mybir
    import concourse.tile as tile
    from concourse import bacc
    from concourse.masks import make_identity

    f32 = mybir.dt.float32
    f32r = mybir.dt.float32r
    bf16 = mybir.dt.bfloat16
    Act = mybir.ActivationFunctionType
    AluMult = mybir.AluOpType.mult
    AluAdd = mybir.AluOpType.add

    assert reps == 1 or reps % 2 == 0

    nc = bacc.Bacc("TRN2", target_bir_lowering=False, debug=False, num_devices=8)
    xT = nc.dram_tensor("xT", [D, L], bf16, kind="ExternalInput")
    wq = nc.dram_tensor("wq", [P, D], bf16, kind="ExternalInput")
    wk = nc.dram_tensor("wk", [P, D], bf16, kind="ExternalInput")
    wv = nc.dram_tensor("wv", [P, D], bf16, kind="ExternalInput")
    wo = nc.dram_tensor("wo", [P, D], bf16, kind="ExternalInput")
    bq = nc.dram_tensor("bq", [P, 1], f32, kind="ExternalInput")
    bk = nc.dram_tensor("bk", [P, 1], f32, kind="ExternalInput")
    out = nc.dram_tensor("out", [L, D], bf16, kind="ExternalOutput")

    with tile.TileContext(nc) as tc:
        with (
            tc.tile_pool(name="const", bufs=1) as constp,
            tc.tile_pool(name="xstream", bufs=16) as xstream,
            tc.tile_pool(name="osbp", bufs=3) as osbp,
            tc.tile_pool(name="expb", bufs=24) as expp,
            tc.tile_pool(name="small", bufs=4) as smallp,
            tc.tile_pool(name="ps_big", bufs=2, space="PSUM") as ps_big,
            tc.tile_pool(name="ps_med", bufs=3, space="PSUM") as ps_med,
            tc.tile_pool(name="ps_av", bufs=2, space="PSUM") as ps_av,
        ):
            # ---------- constants / persistent buffers
            ident = constp.tile([P, P], f32, tag="ident")
            make_identity(nc, ident[:])
            wq_t = constp.tile([P, D], bf16, tag="wq_t")
            wk_t = constp.tile([P, D], bf16, tag="wk_t")
            wv_t = constp.tile([P, D], bf16, tag="wv_t")
            wo_t = constp.tile([P, D], bf16, tag="wo_t")
            for dram, tr in ((wq, wq_t), (wk, wk_t), (wv, wv_t), (wo, wo_t)):
                nc.sync.dma_start(tr[:], dram[:])
            bq_t = constp.tile([P, 1], f32, tag="bq")
            bk_t = constp.tile([P, 1], f32, tag="bk")
            nc.sync.dma_start(bq_t[:], bq[:])
            nc.sync.dma_start(bk_t[:], bk[:])

            nsets = 1 if reps == 1 else 2
            sets = []
            for s in range(nsets):
                st = {
                    "qT": constp.tile([P, L], bf16, tag=f"qT{s}", name=f"qT{s}"),
                    "kT": constp.tile([P, L], bf16, tag=f"kT{s}", name=f"kT{s}"),
                    "vTf": constp.tile([P, L], f32, tag=f"vTf{s}", name=f"vTf{s}"),
                    "vn": constp.tile([P, NB * 130], bf16, tag=f"vn{s}", name=f"vn{s}"),
                    "qg": constp.tile([P, 256], bf16, tag=f"qg{s}", name=f"qg{s}"),
                    "gout": constp.tile([P, 256], bf16, tag=f"gout{s}", name=f"gout{s}"),
                    "id": s,
                }
                for _b in range(NB):
                    nc.gpsimd.memset(st["vn"][:, _b * 130 + 64:_b * 130 + 65], 1.0)
                    nc.gpsimd.memset(st["vn"][:, _b * 130 + 129:_b * 130 + 130], 1.0)
                sets.append(st)

            env = dict(
                constp=constp, xstream=xstream, osbp=osbp, expp=expp,
                smallp=smallp, ps_big=ps_big, ps_med=ps_med, ps_av=ps_av,
                ident=ident, wq_t=wq_t, wk_t=wk_t, wv_t=wv_t, wo_t=wo_t,
                bq_t=bq_t, bk_t=bk_t, xT=xT, out=out,
            )
            if reps == 1:
                _body(nc, mybir, Act, bf16, f32, f32r, AluMult, AluAdd, env, sets[0])
            else:
                with tc.For_i(0, reps // 2, 1):
                    _body(nc, mybir, Act, bf16, f32, f32r, AluMult, AluAdd, env, sets[0])
                    _body(nc, mybir, Act, bf16, f32, f32r, AluMult, AluAdd, env, sets[1])

    nc.compile()
    return nc


def _body(nc, mybir, Act, bf16, f32, f32r, AluMult, AluAdd, env, S):
    xstream = env["xstream"]; osbp = env["osbp"]; expp = env["expp"]; smallp = env["smallp"]
    ps_big = env["ps_big"]; ps_med = env["ps_med"]; ps_av = env["ps_av"]
    ident = env["ident"]; wq_t = env["wq_t"]; wk_t = env["wk_t"]; wv_t = env["wv_t"]; wo_t = env["wo_t"]
    bq_t = env["bq_t"]; bk_t = env["bk_t"]
    xT = env["xT"]; out = env["out"]
    qT = S["qT"]; kT = S["kT"]; vTf = S["vTf"]; vn = S["vn"]
    qg = S["qg"]; gout = S["gout"]; sid = S["id"]

    # ---------- phase A: qkv projections + fused v-transpose, per quad
    def do_quad(quad):
        xrs = []
        for kt in range(8):
            xr = xstream.tile([P, 1024], bf16, tag="xr", bufs=16)
            nc.sync.dma_start(
                xr[:], xT[kt * P:(kt + 1) * P, quad * 1024:(quad + 1) * 1024]
            )
            xrs.append(xr)
        for sub in range(2):
            n = quad * 2 + sub
            sl = slice(n * 512, (n + 1) * 512)
            for wt, dest, bias in (
                (wq_t, qT, bq_t),
                (wk_t, kT, bk_t),
                (wv_t, vTf, None),
            ):
                pp = ps_big.tile([P, 512], f32, tag="psbig")
                for kt in range(8):
                    nc.tensor.matmul(
                        pp[:], wt[:, kt * P:(kt + 1) * P],
                        xrs[kt][:, sub * 512:(sub + 1) * 512],
                        start=kt == 0, stop=kt == 7,
                    )
                if bias is not None:
                    nc.vector.tensor_scalar_add(dest[:, sl], pp[:], bias[:])
                else:
                    nc.vector.tensor_copy(dest[:, sl], pp[:])
        for b in range(8 * quad, 8 * quad + 8):
            pst = ps_av.tile([P, P], f32, tag="psav", name=f"pst{sid}_{b}")
            nc.tensor.transpose(pst[:], vTf[:, b * P:(b + 1) * P], ident[:])
            base = b * 130
            nc.vector.tensor_copy(vn[:, base:base + 64], pst[:, 0:64])
            nc.vector.tensor_copy(vn[:, base + 65:base + 129], pst[:, 64:128])

    def vslice(blk, h):
        return vn[:, blk * 130 + h * 65: blk * 130 + (h + 1) * 65]

    def normalize_emit(psumT, lo, hi, dest):
        # psumT [65, W+]: rows 0:64 = unnormalized outT, row 64 = l.
        # No PE involved: reciprocal (DVE) -> partition_broadcast (Pool)
        # -> columnwise multiply (DVE).
        W = hi - lo
        linv = smallp.tile([1, 512], f32r, tag="linv")
        with nc.allow_low_precision(reason="f32r has near-f32 mantissa here"):
            nc.vector.reciprocal(linv[0:1, 0:W], psumT[64:65, lo:hi])
        bsb = smallp.tile([64, 512], f32r, tag="bsb")
        nc.gpsimd.partition_broadcast(bsb[0:64, 0:W], linv[0:1, 0:W])
        nc.vector.tensor_tensor(dest, psumT[0:64, lo:hi], bsb[0:64, 0:W], AluMult)

    def outproj_tile(stat, tcol, osb, slot):
        # stat [128 hd, 512 q] bf16; project col-tile tcol into osb slot
        for half in (0, 1):
            pso = ps_big.tile([P, 512], f32, tag="psbig")
            nc.tensor.matmul(
                pso[:], stat[:, tcol * P:(tcol + 1) * P],
                wo_t[:, half * 512:(half + 1) * 512],
                start=True, stop=True,
            )
            nc.vector.tensor_copy(
                osb[:, slot * D + half * 512:slot * D + (half + 1) * 512], pso[:]
            )

    def batched_store(osb, slot0, nt, row0):
        # one DMA for nt contiguous 128-row tiles staged in osb
        dst = out[row0 * P:(row0 + nt) * P, :].rearrange(
            "(k p) d -> p k d", p=P
        )
        src = osb[:, slot0 * D:(slot0 + nt) * D].rearrange(
            "p (k d) -> p k d", k=nt
        )
        nc.gpsimd.dma_start(dst, src)

    def chunk_qts(c):
        return [4 * c + i for i in range(4) if (4 * c + i) not in GLOB]

    chunk_exps = {}
    chunk_otr = {}

    # ---------- chunk stage 1: scores + exp (both heads)
    def do_scores(c):
        qts = chunk_qts(c)
        exps = {}
        for h in (0, 1):
            hs = slice(h * 64, (h + 1) * 64)
            egs = {}
            for g in GLOB:
                psg = ps_med.tile([P, 512], f32, tag="psmed")
                nc.tensor.matmul(
                    psg[:], kT[hs, g * P:(g + 1) * P],
                    qT[hs, c * 512:(c + 1) * 512],
                    start=True, stop=True,
                )
                eg = expp.tile([P, 512], bf16, tag="exp")
                nc.scalar.activation(eg[:], psg[:], Act.Exp, scale=SCALE)
                egs[g] = eg
            # diagonal scores: narrow per-qtile matmuls into one psum slab
            psd = ps_med.tile([P, 512], f32, tag="psmed")
            for idx, j in enumerate(qts):
                nc.tensor.matmul(
                    psd[:, idx * P:(idx + 1) * P],
                    kT[hs, j * P:(j + 1) * P], qT[hs, j * P:(j + 1) * P],
                    start=True, stop=True, skip_group_check=True,
                )
            ed = expp.tile([P, 512], bf16, tag="exp")
            nc.scalar.activation(
                ed[:, 0:len(qts) * P], psd[:, 0:len(qts) * P], Act.Exp,
                scale=SCALE,
            )
            exps[h] = (egs, ed)
        chunk_exps[c] = exps

    # ---------- chunk stage 2: AV accumulate + normalize (PE-free norm)
    def do_avnorm(c):
        exps = chunk_exps.pop(c)
        otr = smallp.tile([P, 512], bf16, tag="otr", name=f"otr{sid}_{c}")
        chunk_otr[c] = otr
        lo = 128 if any(g // 4 == c for g in GLOB) else 0
        qts = chunk_qts(c)
        for h in (0, 1):
            egs, ed = exps[h]
            pc = ps_av.tile([65, 512], f32, tag="psav")
            nc.tensor.matmul(pc[:, lo:512], vslice(GLOB[0], h),
                             egs[GLOB[0]][:, lo:512], start=True, stop=False)
            nc.tensor.matmul(pc[:, lo:512], vslice(GLOB[1], h),
                             egs[GLOB[1]][:, lo:512], start=False, stop=True)
            for idx, j in enumerate(qts):
                off = (j - 4 * c) * P
                nc.tensor.matmul(pc[:, off:off + P], vslice(j, h),
                                 ed[:, idx * P:(idx + 1) * P],
                                 start=False, stop=True,
                                 skip_group_check=True)  # sub-region accumulate
            normalize_emit(pc, lo, 512, otr[h * 64:(h + 1) * 64, lo:512])

    # ---------- chunk stage 3: out-projection + one batched store
    def do_outproj(c):
        otr = chunk_otr.pop(c)
        qts = chunk_qts(c)
        osb = osbp.tile([P, 4 * D], bf16, tag="osb")
        for j in qts:
            outproj_tile(otr, j - 4 * c, osb, j - 4 * c)
        batched_store(osb, qts[0] - 4 * c, len(qts), qts[0])

    # ---------- global qtiles (0 and 24): attend to all 32 blocks,
    # software-pipelined so PE never waits on an exp round-trip.
    def do_global():
        pgs = {}
        for h in (0, 1):
            pgs[h] = ps_med.tile([65, 256], f32, tag="gt", bufs=1,
                                 name=f"pg{sid}_{h}")

        def g_av(h, kb2, eg):
            for half in (0, 1):
                kb = 2 * kb2 + half
                nc.tensor.matmul(
                    pgs[h][:], vslice(kb, h), eg[:, half * 256:(half + 1) * 256],
                    start=kb == 0, stop=kb == NB - 1,
                )

        pend = []
        for kb2 in range(NB // 2):
            for h in (0, 1):
                hs = slice(h * 64, (h + 1) * 64)
                psg = ps_med.tile([P, 512], f32, tag="psmed")
                for half in (0, 1):
                    kb = 2 * kb2 + half
                    nc.tensor.matmul(
                        psg[:, half * 256:(half + 1) * 256],
                        kT[hs, kb * P:(kb + 1) * P], qg[hs, :],
                        start=True, stop=True,
                    )
                eg = expp.tile([P, 512], bf16, tag="exp")
                nc.scalar.activation(eg[:], psg[:], Act.Exp, scale=SCALE)
                pend.append((h, kb2, eg))
                if len(pend) >= 3:
                    g_av(*pend.pop(0))
        while pend:
            g_av(*pend.pop(0))
        for h in (0, 1):
            normalize_emit(pgs[h], 0, 256, gout[h * 64:(h + 1) * 64, :])

    def do_outproj_global():
        osb = osbp.tile([P, 4 * D], bf16, tag="osb")
        for ti, g in enumerate(GLOB):
            outproj_tile(gout, ti, osb, ti)
            batched_store(osb, ti, 1, g)

    # ---------- schedule: chunk stages pipelined 1 apart so every PE
    # stall (exp round-trip, DVE normalize) is covered by other work.
    do_quad(0)
    do_quad(3)
    # stage global-q columns (available after quads 0 and 3)
    nc.gpsimd.tensor_copy(qg[:, 0:128], qT[:, 0:128])
    nc.gpsimd.tensor_copy(qg[:, 128:256], qT[:, GLOB[1] * P:(GLOB[1] + 1) * P])
    do_scores(1)
    do_quad(1)
    do_scores(0)
    do_quad(2)
    do_scores(2)
    do_avnorm(1)
    do_scores(3)
    do_avnorm(0)
    do_outproj(1)
    do_global()
    do_scores(4)
    do_avnorm(2)
    do_outproj(0)
    do_outproj_global()
    do_scores(5)
    do_avnorm(3)
    do_outproj(2)
    do_scores(7)
    do_avnorm(4)
    do_outproj(3)
    do_scores(6)
    do_avnorm(5)
    do_outproj(4)
    do_avnorm(7)
    do_outproj(5)
    do_avnorm(6)
    do_outproj(7)
    do_outproj(6)


def _get_nc(reps=1):
    key = ("nc", reps)
    if key not in _CACHE:
        _CACHE[key] = _build_nc(reps)
    return _CACHE[key]


def _prep_inputs(x, w_qkv, b_qkv):
    import ml_dtypes

    bf = ml_dtypes.bfloat16
    x2 = np.asarray(x, dtype=np.float32).reshape(L, D)
    xT = np.ascontiguousarray(x2.T.astype(bf))
    w_qkv = np.asarray(w_qkv, dtype=np.float32)
    b_qkv = np.asarray(b_qkv, dtype=np.float32)

    def tile_w(w_slice):
        wt = w_slice.T
        return np.ascontiguousarray(
            wt.reshape(8, P, P).transpose(1, 0, 2).reshape(P, D).astype(bf)
        )

    maps = []
    for c in range(8):
        a = 2 * c * HD
        b = a + 2 * HD
        maps.append({
            "xT": xT,
            "wq": tile_w(w_qkv[a:b, :]),
            "wk": tile_w(w_qkv[D + a:D + b, :]),
            "wv": tile_w(w_qkv[2 * D + a:2 * D + b, :]),
            "bq": np.ascontiguousarray(b_qkv[a:b].reshape(P, 1)),
            "bk": np.ascontiguousarray(b_qkv[D + a:D + b].reshape(P, 1)),
        })
    return maps


def kernel(x, w_qkv, b_qkv, w_out, b_out):
    import ml_dtypes
    from concourse.bass_utils import run_bass_kernel_spmd

    bf = ml_dtypes.bfloat16
    x = np.asarray(x, dtype=np.float32)
    w_qkv = np.asarray(w_qkv, dtype=np.float32)
    b_qkv = np.asarray(b_qkv, dtype=np.float32)
    w_out = np.asarray(w_out, dtype=np.float32)
    b_out = np.asarray(b_out, dtype=np.float32)

    nc = _get_nc()
    maps = _prep_inputs(x, w_qkv, b_qkv)
    for c in range(8):
        a = 2 * c * HD
        b = a + 2 * HD
        maps[c]["wo"] = np.ascontiguousarray(w_out[:, a:b].T.astype(bf))

    res = run_bass_kernel_spmd(nc, maps, core_ids=list(range(8)))

    total = res.results[0]["out"].astype(np.float32)
    for c in range(1, 8):
        total += res.results[c]["out"].astype(np.float32)
    const_row = b_qkv[2 * D:3 * D] @ w_out.T + b_out
    total += const_row[None, :]
    return total.reshape(x.shape).astype(np.float32)


# revision 28
# speedup vs baseline: 1.0271x; 1.0271x over previous
"""Block-sparse attention Trainium2 kernel (v3, bf16 + A/B pipelined bodies).

Reference: nn.MultiheadAttention-style block-sparse attention, B=1, L=4096,
D=1024, H=16, head_dim=64, block=128, global blocks {0, 24}.

Sharding: head-parallel across 8 cores (2 heads/core); host sums the 8
partial out-projections. All matmul operands are bf16 (PSUM accumulates
f32), which halves HBM traffic (x in, partial-out store) and runs narrow
(128-wide) matmuls at 1 cycle/row. Attention-value products are computed
in transposed form (outT = v_aug.T @ expT); softmax denominators ride
along as row 64 of the augmented V. For steady-state timing the loop body
is unrolled 2x over two independent buffer sets (A/B) so consecutive
bodies pipeline with no WAR hazards on q/k/v. The Act engine runs only
Exp (no act-table swaps); PSUM->SBUF copies are spread over DVE/Pool.
"""

import sys

sys.path.insert(0, "/opt/trn_rl_repo")
import numpy as np

D = 1024
L = 4096
H = 16
HD = 64
NB = 32
GLOB = (0, 24)
P = 128
SCALE = 1.0 / 8.0

_CACHE = {}


def _build_nc(reps=1):
    import contextlib

    import concourse.mybir as mybir
    import concourse.tile as tile
    from concourse import bacc
    from concourse.masks import make_identity

    f32 = mybir.dt.float32
    f32r = mybir.dt.float32r
    bf16 = mybir.dt.bfloat16
    Act = mybir.ActivationFunctionType
    AluMult = mybir.AluOpType.mult
    AluAdd = mybir.AluOpType.add

    assert reps == 1 or reps % 2 == 0

    nc = bacc.Bacc("TRN2", target_bir_lowering=False, debug=False, num_devices=8)
    xT = nc.dram_tensor("xT", [D, L], bf16, kind="ExternalInput")
    wq = nc.dram_tensor("wq", [P, D], bf16, kind="ExternalInput")
    wk = nc.dram_tensor("wk", [P, D], bf16, kind="ExternalInput")
    wv = nc.dram_tensor("wv", [P, D], bf16, kind="ExternalInput")
    wo = nc.dram_tensor("wo", [P, D], bf16, kind="ExternalInput")
    bq = nc.dram_tensor("bq", [P, 1], f32, kind="ExternalInput")
    bk = nc.dram_tensor("bk", [P, 1], f32, kind="ExternalInput")
    out = nc.dram_tensor("out", [L, D], bf16, kind="ExternalOutput")

    with tile.TileContext(nc) as tc:
        with (
            tc.tile_pool(name="const", bufs=1) as constp,
            tc.tile_pool(name="xstream", bufs=16) as xstream,
            tc.tile_pool(name="osbp", bufs=3) as osbp,
            tc.tile_pool(name="expb", bufs=18) as expp,
            tc.tile_pool(name="small", bufs=4) as smallp,
            tc.tile_pool(name="ps_big", bufs=2, space="PSUM") as ps_big,
            tc.tile_pool(name="ps_med", bufs=3, space="PSUM") as ps_med,
            tc.tile_pool(name="ps_av", bufs=2, space="PSUM") as ps_av,
        ):
            # ---------- constants / persistent buffers
            ident = constp.tile([P, P], f32, tag="ident")
            make_identity(nc, ident[:])
            wq_t = constp.tile([P, D], bf16, tag="wq_t")
            wk_t = constp.tile([P, D], bf16, tag="wk_t")
            wv_t = constp.tile([P, D], bf16, tag="wv_t")
            wo_t = constp.tile([P, D], bf16, tag="wo_t")
            for dram, tr in ((wq, wq_t), (wk, wk_t), (wv, wv_t), (wo, wo_t)):
                nc.sync.dma_start(tr[:], dram[:])
            bq_t = constp.tile([P, 1], f32, tag="bq")
            bk_t = constp.tile([P, 1], f32, tag="bk")
            nc.sync.dma_start(bq_t[:], bq[:])
            nc.sync.dma_start(bk_t[:], bk[:])

            nsets = 1 if reps == 1 else 2
            sets = []
            for s in range(nsets):
                st = {
                    "qT": constp.tile([P, L], bf16, tag=f"qT{s}", name=f"qT{s}"),
                    "kT": constp.tile([P, L], bf16, tag=f"kT{s}", name=f"kT{s}"),
                    "vTf": constp.tile([P, L], f32, tag=f"vTf{s}", name=f"vTf{s}"),
                    "vn": constp.tile([P, NB * 130], bf16, tag=f"vn{s}", name=f"vn{s}"),
                    "qg": constp.tile([P, 256], bf16, tag=f"qg{s}", name=f"qg{s}"),
                    "gout": constp.tile([P, 256], bf16, tag=f"gout{s}", name=f"gout{s}"),
                    "id": s,
                }
                for _b in range(NB):
                    nc.gpsimd.memset(st["vn"][:, _b * 130 + 64:_b * 130 + 65], 1.0)
                    nc.gpsimd.memset(st["vn"][:, _b * 130 + 129:_b * 130 + 130], 1.0)
                sets.append(st)

            env = dict(
                constp=constp, xstream=xstream, osbp=osbp, expp=expp,
                smallp=smallp, ps_big=ps_big, ps_med=ps_med, ps_av=ps_av,
                ident=ident, wq_t=wq_t, wk_t=wk_t, wv_t=wv_t, wo_t=wo_t,
                bq_t=bq_t, bk_t=bk_t, xT=xT, out=out,
            )
            if reps == 1:
                _body(nc, mybir, Act, bf16, f32, f32r, AluMult, AluAdd, env, sets[0])
            else:
                with tc.For_i(0, reps // 2, 1):
                    _body(nc, mybir, Act, bf16, f32, f32r, AluMult, AluAdd, env, sets[0])
                    _body(nc, mybir, Act, bf16, f32, f32r, AluMult, AluAdd, env, sets[1])

    nc.compile()
    return nc


def _body(nc, mybir, Act, bf16, f32, f32r, AluMult, AluAdd, env, S):
    xstream = env["xstream"]; osbp = env["osbp"]; expp = env["expp"]; smallp = env["smallp"]
    ps_big = env["ps_big"]; ps_med = env["ps_med"]; ps_av = env["ps_av"]
    ident = env["ident"]; wq_t = env["wq_t"]; wk_t = env["wk_t"]; wv_t = env["wv_t"]; wo_t = env["wo_t"]
    bq_t = env["bq_t"]; bk_t = env["bk_t"]
    xT = env["xT"]; out = env["out"]
    qT = S["qT"]; kT = S["kT"]; vTf = S["vTf"]; vn = S["vn"]
    qg = S["qg"]; gout = S["gout"]; sid = S["id"]

    # ---------- phase A: qkv projections + fused v-transpose, per quad
    def do_quad(quad):
        xrs = []
        for kt in range(8):
            xr = xstream.tile([P, 1024], bf16, tag="xr", bufs=16)
            nc.sync.dma_start(
                xr[:], xT[kt * P:(kt + 1) * P, quad * 1024:(quad + 1) * 1024]
            )
            xrs.append(xr)
        for sub in range(2):
            n = quad * 2 + sub
            sl = slice(n * 512, (n + 1) * 512)
            for wt, dest, bias in (
                (wq_t, qT, bq_t),
                (wk_t, kT, bk_t),
                (wv_t, vTf, None),
            ):
                pp = ps_big.tile([P, 512], f32, tag="psbig")
                for kt in range(8):
                    nc.tensor.matmul(
                        pp[:], wt[:, kt * P:(kt + 1) * P],
                        xrs[kt][:, sub * 512:(sub + 1) * 512],
                        start=kt == 0, stop=kt == 7,
                    )
                if bias is not None:
                    nc.vector.tensor_scalar_add(dest[:, sl], pp[:], bias[:])
                else:
                    nc.vector.tensor_copy(dest[:, sl], pp[:])
        for b in range(8 * quad, 8 * quad + 8):
            pst = ps_av.tile([P, P], f32, tag="psav", name=f"pst{sid}_{b}")
            nc.tensor.transpose(pst[:], vTf[:, b * P:(b + 1) * P], ident[:])
            base = b * 130
            nc.vector.tensor_copy(vn[:, base:base + 64], pst[:, 0:64])
            nc.vector.tensor_copy(vn[:, base + 65:base + 129], pst[:, 64:128])

    def vslice(blk, h):
        return vn[:, blk * 130 + h * 65: blk * 130 + (h + 1) * 65]

    def normalize_emit(psumT, lo, hi, dest):
        # psumT [65, W+]: rows 0:64 = unnormalized outT, row 64 = l.
        # No PE involved: reciprocal (DVE) -> partition_broadcast (Pool)
        # -> columnwise multiply (DVE).
        W = hi - lo
        linv = smallp.tile([1, 512], f32r, tag="linv")
        with nc.allow_low_precision(reason="f32r has near-f32 mantissa here"):
            nc.vector.reciprocal(linv[0:1, 0:W], psumT[64:65, lo:hi])
        bsb = smallp.tile([64, 512], f32r, tag="bsb")
        nc.gpsimd.partition_broadcast(bsb[0:64, 0:W], linv[0:1, 0:W])
        nc.vector.tensor_tensor(dest, psumT[0:64, lo:hi], bsb[0:64, 0:W], AluMult)

    def outproj_tile(stat, tcol, osb, slot):
        # stat [128 hd, 512 q] bf16; project col-tile tcol into osb slot
        for half in (0, 1):
            pso = ps_big.tile([P, 512], f32, tag="psbig")
            nc.tensor.matmul(
                pso[:], stat[:, tcol * P:(tcol + 1) * P],
                wo_t[:, half * 512:(half + 1) * 512],
                start=True, stop=True,
            )
            nc.vector.tensor_copy(
                osb[:, slot * D + half * 512:slot * D + (half + 1) * 512], pso[:]
            )

    def batched_store(osb, slot0, nt, row0):
        # one DMA for nt contiguous 128-row tiles staged in osb
        dst = out[row0 * P:(row0 + nt) * P, :].rearrange(
            "(k p) d -> p k d", p=P
        )
        src = osb[:, slot0 * D:(slot0 + nt) * D].rearrange(
            "p (k d) -> p k d", k=nt
        )
        nc.gpsimd.dma_start(dst, src)

    def chunk_qts(c):
        return [4 * c + i for i in range(4) if (4 * c + i) not in GLOB]

    chunk_exps = {}
    chunk_otr = {}

    # ---------- chunk stage 1: scores + exp (both heads)
    def do_scores(c):
        qts = chunk_qts(c)
        exps = {}
        for h in (0, 1):
            hs = slice(h * 64, (h + 1) * 64)
            egs = {}
            for g in GLOB:
                psg = ps_med.tile([P, 512], f32, tag="psmed")
                nc.tensor.matmul(
                    psg[:], kT[hs, g * P:(g + 1) * P],
                    qT[hs, c * 512:(c + 1) * 512],
                    start=True, stop=True,
                )
                eg = expp.tile([P, 512], bf16, tag="exp")
                nc.scalar.activation(eg[:], psg[:], Act.Exp, scale=SCALE)
                egs[g] = eg
            # diagonal scores: narrow per-qtile matmuls into one psum slab
            psd = ps_med.tile([P, 512], f32, tag="psmed")
            for idx, j in enumerate(qts):
                nc.tensor.matmul(
                    psd[:, idx * P:(idx + 1) * P],
                    kT[hs, j * P:(j + 1) * P], qT[hs, j * P:(j + 1) * P],
                    start=True, stop=True, skip_group_check=True,
                )
            ed = expp.tile([P, 512], bf16, tag="exp")
            nc.scalar.activation(
                ed[:, 0:len(qts) * P], psd[:, 0:len(qts) * P], Act.Exp,
                scale=SCALE,
            )
            exps[h] = (egs, ed)
        chunk_exps[c] = exps

    # ---------- chunk stage 2: AV accumulate + normalize (PE-free norm)
    def do_avnorm(c):
        exps = chunk_exps.pop(c)
        otr = smallp.tile([P, 512], bf16, tag="otr", name=f"otr{sid}_{c}")
        chunk_otr[c] = otr
        lo = 128 if any(g // 4 == c for g in GLOB) else 0
        qts = chunk_qts(c)
        for h in (0, 1):
            egs, ed = exps[h]
            pc = ps_av.tile([65, 512], f32, tag="psav")
            nc.tensor.matmul(pc[:, lo:512], vslice(GLOB[0], h),
                             egs[GLOB[0]][:, lo:512], start=True, stop=False)
            nc.tensor.matmul(pc[:, lo:512], vslice(GLOB[1], h),
                             egs[GLOB[1]][:, lo:512], start=False, stop=True)
            for idx, j in enumerate(qts):
                off = (j - 4 * c) * P
                nc.tensor.matmul(pc[:, off:off + P], vslice(j, h),
                                 ed[:, idx * P:(idx + 1) * P],
                                 start=False, stop=True,
                                 skip_group_check=True)  # sub-region accumulate
            normalize_emit(pc, lo, 512, otr[h * 64:(h + 1) * 64, lo:512])

    # ---------- chunk stage 3: out-projection + one batched store
    def do_outproj(c):
        otr = chunk_otr.pop(c)
        qts = chunk_qts(c)
        osb = osbp.tile([P, 4 * D], bf16, tag="osb")
        for j in qts:
            outproj_tile(otr, j - 4 * c, osb, j - 4 * c)
        batched_store(osb, qts[0] - 4 * c, len(qts), qts[0])

    # ---------- global qtiles (0 and 24): attend to all 32 blocks,
    # software-pipelined so PE never waits on an exp round-trip.
    def do_global():
        pgs = {}
        for h in (0, 1):
            pgs[h] = ps_med.tile([65, 256], f32, tag="gt", bufs=1,
                                 name=f"pg{sid}_{h}")

        def g_av(h, kb2, eg):
            for half in (0, 1):
                kb = 2 * kb2 + half
                nc.tensor.matmul(
                    pgs[h][:], vslice(kb, h), eg[:, half * 256:(half + 1) * 256],
                    start=kb == 0, stop=kb == NB - 1,
                )

        pend = []
        for kb2 in range(NB // 2):
            for h in (0, 1):
                hs = slice(h * 64, (h + 1) * 64)
                psg = ps_med.tile([P, 512], f32, tag="psmed")
                for half in (0, 1):
                    kb = 2 * kb2 + half
                    nc.tensor.matmul(
                        psg[:, half * 256:(half + 1) * 256],
                        kT[hs, kb * P:(kb + 1) * P], qg[hs, :],
                        start=True, stop=True,
                    )
                eg = expp.tile([P, 512], bf16, tag="exp")
                nc.scalar.activation(eg[:], psg[:], Act.Exp, scale=SCALE)
                pend.append((h, kb2, eg))
                if len(pend) >= 3:
                    g_av(*pend.pop(0))
        while pend:
            g_av(*pend.pop(0))
        for h in (0, 1):
            normalize_emit(pgs[h], 0, 256, gout[h * 64:(h + 1) * 64, :])

    def do_outproj_global():
        osb = osbp.tile([P, 4 * D], bf16, tag="osb")
        for ti, g in enumerate(GLOB):
            outproj_tile(gout, ti, osb, ti)
            batched_store(osb, ti, 1, g)

    # ---------- schedule: chunk stages pipelined 1 apart so every PE
    # stall (exp round-trip, DVE normalize) is covered by other work.
    do_quad(0)
    do_quad(3)
    # stage global-q columns (available after quads 0 and 3)
    nc.gpsimd.tensor_copy(qg[:, 0:128], qT[:, 0:128])
    nc.gpsimd.tensor_copy(qg[:, 128:256], qT[:, GLOB[1] * P:(GLOB[1] + 1) * P])
    do_scores(1)
    do_quad(1)
    do_scores(0)
    do_avnorm(1)
    do_quad(2)
    do_scores(2)
    do_avnorm(0)
    do_outproj(1)
    do_scores(3)
    do_avnorm(2)
    do_outproj(0)
    do_global()
    do_scores(4)
    do_avnorm(3)
    do_outproj_global()
    do_outproj(2)
    do_scores(5)
    do_avnorm(4)
    do_outproj(3)
    do_scores(7)
    do_avnorm(5)
    do_outproj(4)
    do_scores(6)
    do_avnorm(7)
    do_outproj(5)
    do_avnorm(6)
    do_outproj(7)
    do_outproj(6)


def _get_nc(reps=1):
    key = ("nc", reps)
    if key not in _CACHE:
        _CACHE[key] = _build_nc(reps)
    return _CACHE[key]


def _prep_inputs(x, w_qkv, b_qkv):
    import ml_dtypes

    bf = ml_dtypes.bfloat16
    x2 = np.asarray(x, dtype=np.float32).reshape(L, D)
    xT = np.ascontiguousarray(x2.T.astype(bf))
    w_qkv = np.asarray(w_qkv, dtype=np.float32)
    b_qkv = np.asarray(b_qkv, dtype=np.float32)

    def tile_w(w_slice):
        wt = w_slice.T
        return np.ascontiguousarray(
            wt.reshape(8, P, P).transpose(1, 0, 2).reshape(P, D).astype(bf)
        )

    maps = []
    for c in range(8):
        a = 2 * c * HD
        b = a + 2 * HD
        maps.append({
            "xT": xT,
            "wq": tile_w(w_qkv[a:b, :]),
            "wk": tile_w(w_qkv[D + a:D + b, :]),
            "wv": tile_w(w_qkv[2 * D + a:2 * D + b, :]),
            "bq": np.ascontiguousarray(b_qkv[a:b].reshape(P, 1)),
            "bk": np.ascontiguousarray(b_qkv[D + a:D + b].reshape(P, 1)),
        })
    return maps


def kernel(x, w_qkv, b_qkv, w_out, b_out):
    import ml_dtypes
    from concourse.bass_utils import run_bass_kernel_spmd

    bf = ml_dtypes.bfloat16
    x = np.asarray(x, dtype=np.float32)
    w_qkv = np.asarray(w_qkv, dtype=np.float32)
    b_qkv = np.asarray(b_qkv, dtype=np.float32)
    w_out = np.asarray(w_out, dtype=np.float32)
    b_out = np.asarray(b_out, dtype=np.float32)

    nc = _get_nc()
    maps = _prep_inputs(x, w_qkv, b_qkv)
    for c in range(8):
        a = 2 * c * HD
        b = a + 2 * HD
        maps[c]["wo"] = np.ascontiguousarray(w_out[:, a:b].T.astype(bf))

    res = run_bass_kernel_spmd(nc, maps, core_ids=list(range(8)))

    total = res.results[0]["out"].astype(np.float32)
    for c in range(1, 8):
        total += res.results[c]["out"].astype(np.float32)
    const_row = b_qkv[2 * D:3 * D] @ w_out.T + b_out
    total += const_row[None, :]
    return total.reshape(x.shape).astype(np.float32)
